# revision 6
# baseline (speedup 1.0000x reference)
"""Trainium2 8-core kernel for modality-routed attention (nn_Attention_21715354648747).

Strategy (per sharding hint + modality-sorted routing):
- Host: sort tokens by modality; fold pre_norm into qkv_w, fold q/k_norm+rope
  into per-token tables; pre-tile weights per core (column-split QKV: 5 Q heads
  + 1 KV group + gates per core; row-split proj over the core's 640 head dims).
- Device: seq-parallel RMSNorm + transpose -> AllGather(bf16) -> routed QKV
  (weights stationary, per-modality contiguous token segments) -> QK norm +
  rope in [d,t] layout -> attention per head (scores^T, exp, PV + denominator
  matmuls) -> gated, 1/denom scaled -> routed proj -> ReduceScatter(bf16).
- Host: concat shards, invert permutation, cast f32.
"""
import sys

for _p in ("/opt/trn_rl_repo",):
    if _p not in sys.path:
        sys.path.append(_p)

import numpy as np
import ml_dtypes

import concourse.bass as bass
import concourse.tile as tile
from concourse import mybir
from concourse.masks import make_identity

# ---------------- problem constants (hardcoded) ----------------
S = 2048
H = 5120
HB = H // 128          # 40 h-blocks
D = 128
NCORE = 8
SLOC = S // NCORE      # 256 tokens per core
NQH = 40
NHL = NQH // NCORE     # 5 q heads per core
NKV = 8
M = 3
QKV_PER_MOD = NQH * D + 2 * NKV * D + NQH  # 7208
Q_DIM = NQH * D        # 5120
K_DIM = NKV * D        # 1024
EPS = 1e-6

BF = mybir.dt.bfloat16
F32 = mybir.dt.float32
AF = mybir.ActivationFunctionType
P = 128

_MAX_WAITS = 1
_wsplit_counter = [0]


def _split_excess_waits(nc, max_waits=_MAX_WAITS):
    """This walrus build encodes at most one sync wait per instruction; Tile's
    wait coalescing (notably the kernel-tail Drain) can exceed that. Move the
    excess waits onto NOPs inserted immediately before, on the same engine."""
    for fn in nc.m.functions:
        for bb in fn.blocks:
            il = bb.instructions
            snapshot = list(il)
            new_list = []
            changed = False
            for ins in snapshot:
                si = ins.sync_info
                waits = list(si.on_wait) if si is not None else []
                if len(waits) > max_waits:
                    extra = waits[: len(waits) - max_waits]
                    keep = waits[len(waits) - max_waits:]
                    for c in range(0, len(extra), max_waits):
                        chunk = extra[c:c + max_waits]
                        _wsplit_counter[0] += 1
                        nop = mybir.InstNoOp(
                            name=f"Wsplit-{_wsplit_counter[0]}", ins=[], outs=[]
                        )
                        nop.engine = ins.engine
                        nop.sync_info = mybir.SyncInfo(on_wait=chunk, on_update=[])
                        new_list.append(nop)
                        changed = True
                    si.on_wait[:] = keep
                new_list.append(ins)
            if changed:
                il[:] = new_list


def _chunks(lo, hi, step):
    out = []
    t = lo
    while t < hi:
        out.append((t, min(t + step, hi)))
        t += step
    return out


def build_module(counts, segw):
    """Build the SPMD Bass module for given modality counts (token-sorted)."""
    c0, c1, c2 = counts
    off = [0, c0, c0 + c1, S]
    nc = bass.Bass()

    # ---- DRAM parameters (per-core shards via in_maps) ----
    xs_ext = nc.declare_dram_parameter("xs", [SLOC, H], F32, isOutput=False)
    qkvw_ext = nc.declare_dram_parameter("qkvw", [M, 8, P, HB, P], BF, isOutput=False)
    projw_ext = nc.declare_dram_parameter("projw", [M, 10, P, NHL, 512], BF, isOutput=False)
    ropes_ext = nc.declare_dram_parameter("ropes", [4, P, S], BF, isOutput=False)
    bmask_ext = nc.declare_dram_parameter("bmask", [4, P, 1], F32, isOutput=False)
    out_ext = nc.declare_dram_parameter("out", [2, SLOC, 2560], BF, isOutput=True)

    # ---- internal DRAM ----
    agin = nc.dram_tensor("agin", [HB, P, SLOC], BF)
    agout = nc.dram_tensor("agout", [NCORE, HB, P, SLOC], BF, addr_space="Shared")
    gsd = nc.dram_tensor("gsd", [NHL, S], BF)
    y0 = nc.dram_tensor("y0", [S, 2560], BF)
    y1 = nc.dram_tensor("y1", [S, 2560], BF)
    rs0 = nc.dram_tensor("rs0", [SLOC, 2560], BF)
    rs1 = nc.dram_tensor("rs1", [SLOC, 2560], BF)

    RG = [list(range(NCORE))]

    # boundary chunks for proj (tokens on partitions, chunks of 128)
    bnds = {}  # tc -> boundary idx (0: between mod0/1, 1: between mod1/2)
    for b in (1, 2):
        if off[b] % P != 0:
            bnds[off[b] // P] = b - 1

    with tile.TileContext(nc) as tc:
        with tc.tile_pool(name="const", bufs=1) as constp, \
             tc.tile_pool(name="resident", bufs=1) as resp:
            identb = constp.tile([P, P], BF)
            make_identity(nc, identb[:])
            ones_b = constp.tile([P, 1], BF)
            nc.vector.memset(ones_b[:], 1.0)
            ropes = constp.tile([P, 4 * S], BF)
            nc.sync.dma_start(
                ropes[:].rearrange("p (a f) -> p a f", a=4),
                ropes_ext.rearrange("a p f -> p a f"))
            bmask = constp.tile([P, 4], F32)
            nc.sync.dma_start(
                bmask[:].rearrange("p (a f) -> p a f", a=4),
                bmask_ext.rearrange("a p f -> p a f"))

            # qkvT resident tiles: 0-4 q heads, 5 k, 6 v, 7 gates (then v_nat)
            qkvT = [resp.tile([P, S], BF, tag=f"qkvT{i}", name=f"qkvT{i}") for i in range(8)]
            v_nat = qkvT[7]  # reused after gates move to DRAM
            ogt = qkvT[:NHL]  # roped q overwritten by gated attn out slices

            # ---------------- phase A: pre-norm + transpose + AllGather ----
            with tc.tile_pool(name="phA", bufs=2) as pa, \
                 tc.tile_pool(name="phA_st", bufs=1) as pst, \
                 tc.tile_pool(name="phA_ps", bufs=3, space="PSUM") as paps:
                stage = [pst.tile([P, SLOC], BF, tag=f"stage{hb}", name=f"stage{hb}") for hb in range(HB)]
                for tt in range(SLOC // P):
                    xt = pa.tile([P, H], F32, tag="xt")
                    nc.sync.dma_start(xt[:], xs_ext[tt * P:(tt + 1) * P, :])
                    sq = pa.tile([P, H], BF, tag="sq")
                    ssq = pa.tile([P, 1], F32, tag="ssq")
                    nc.scalar.activation(sq[:], xt[:], AF.Square, accum_out=ssq[:])
                    z = pa.tile([P, 1], F32, tag="z")
                    nc.vector.tensor_scalar(z[:], ssq[:], 1.0 / H, EPS,
                                            mybir.AluOpType.mult, mybir.AluOpType.add)
                    zr = pa.tile([P, 1], F32, tag="zr")
                    nc.vector.reciprocal(zr[:], z[:])
                    inv = pa.tile([P, 1], F32, tag="inv")
                    nc.scalar.activation(inv[:], zr[:], AF.Sqrt)
                    xh = pa.tile([P, H], BF, tag="xh")
                    nc.vector.tensor_scalar_mul(xh[:], xt[:], inv[:])
                    for hb in range(HB):
                        tp = paps.tile([P, P], BF, tag="tp")
                        nc.tensor.transpose(tp[:], xh[:, hb * P:(hb + 1) * P], identb[:])
                        nc.vector.tensor_copy(stage[hb][:, tt * P:(tt + 1) * P], tp[:])
                for hb in range(HB):
                    nc.sync.dma_start(agin[hb], stage[hb][:])
            nc.gpsimd.collective_compute(
                "AllGather", mybir.AluOpType.bypass, replica_groups=RG,
                ins=[agin[:]], outs=[agout[:]])

            # ---------------- phase B: routed QKV ----------------
            with tc.tile_pool(name="phB_x", bufs=1) as pbx, \
                 tc.tile_pool(name="phB_w", bufs=2) as pbw, \
                 tc.tile_pool(name="phB_e", bufs=3) as pbe, \
                 tc.tile_pool(name="phB_ps", bufs=4, space="PSUM") as pbps:
                xseg = pbx.tile([P, HB * segw], BF)
                for m in range(M):
                    lo, hi = off[m], off[m + 1]
                    cnt = hi - lo
                    r0, r1 = lo // SLOC, (hi - 1) // SLOC
                    for r in range(r0, r1 + 1):
                        glo, ghi = max(lo, r * SLOC), min(hi, (r + 1) * SLOC)
                        w = ghi - glo
                        dst = glo - lo
                        nc.sync.dma_start(
                            xseg[:].rearrange("p (hb f) -> p hb f", hb=HB)[
                                :, :, dst:dst + w],
                            agout[r].rearrange("hb p f -> p hb f")[
                                :, :, glo - r * SLOC:ghi - r * SLOC])
                    for ot in range(8):
                        wbuf = pbw.tile([P, HB * P], BF, tag="wbuf")
                        nc.sync.dma_start(
                            wbuf[:].rearrange("p (hb f) -> p hb f", hb=HB),
                            qkvw_ext[m, ot])
                        for (tl, th) in _chunks(0, cnt, 512):
                            w = th - tl
                            ps = pbps.tile([P, 512], F32, tag="qkvps")
                            for hb in range(HB):
                                nc.tensor.matmul(
                                    ps[:, :w],
                                    wbuf[:, hb * P:(hb + 1) * P],
                                    xseg[:, hb * segw + tl:hb * segw + th],
                                    start=(hb == 0), stop=(hb == HB - 1))
                            nc.vector.tensor_copy(
                                qkvT[ot][:, lo + tl:lo + th], ps[:, :w])

            # ---------------- phase C: gates out; q/k norm + rope; v_nat ----
            with tc.tile_pool(name="phC", bufs=2) as pc, \
                 tc.tile_pool(name="phC_ps", bufs=2, space="PSUM") as pcps, \
                 tc.tile_pool(name="phC_dram", bufs=2, space="DRAM") as pcd:
                # gates: sigmoid then rows to DRAM (qkvT[7] freed for v_nat)
                grows = pc.tile([NHL, S], BF, tag="grows")
                nc.scalar.activation(grows[:], qkvT[7][0:NHL, :], AF.Sigmoid)
                nc.sync.dma_start(gsd[:], grows[:])

                for kk in range(NHL + 1):  # 0..4 q heads, 5 = k
                    src = qkvT[kk]
                    is_q = kk < NHL
                    sq = pc.tile([P, S], BF, tag="csq")
                    nc.scalar.activation(sq[:], src[:], AF.Square)
                    invrow = pc.tile([1, S], BF, tag="invrow")
                    for ic in range(S // 512):
                        ssp = pcps.tile([1, 512], F32, tag="ssp")
                        nc.tensor.matmul(ssp[:], ones_b[:],
                                         sq[:, ic * 512:(ic + 1) * 512],
                                         start=True, stop=True)
                        z = pc.tile([1, 512], F32, tag="cz")
                        if is_q:
                            # fold 1/sqrt(D): rsqrt(ssq + D*eps)
                            nc.vector.tensor_scalar_add(z[:], ssp[:], D * EPS)
                        else:
                            nc.vector.tensor_scalar(z[:], ssp[:], 1.0 / D, EPS,
                                                    mybir.AluOpType.mult,
                                                    mybir.AluOpType.add)
                        zr = pc.tile([1, 512], F32, tag="czr")
                        nc.vector.reciprocal(zr[:], z[:])
                        nc.scalar.activation(invrow[:, ic * 512:(ic + 1) * 512],
                                             zr[:], AF.Sqrt)
                    invd = pcd.tile([1, S], BF, tag="invd")
                    nc.sync.dma_start(invd[:], invrow[:])
                    invb = pc.tile([P, S], BF, tag="invb")
                    nc.sync.dma_start(invb[:], invd[0:1, :].to_broadcast([P, S]))
                    sh = pc.tile([P, S], BF, tag="csh")
                    nc.sync.dma_start(sh[0:64, :], src[64:128, :])
                    nc.sync.dma_start(sh[64:128, :], src[0:64, :])
                    A = ropes[:, (0 if is_q else 2) * S:(1 if is_q else 3) * S]
                    B = ropes[:, (1 if is_q else 3) * S:(2 if is_q else 4) * S]
                    t1 = pc.tile([P, S], BF, tag="ct1")
                    nc.vector.tensor_mul(t1[:], src[:], A)
                    t2 = pc.tile([P, S], BF, tag="ct2")
                    nc.vector.tensor_mul(t2[:], sh[:], B)
                    t3 = pc.tile([P, S], BF, tag="ct3")
                    nc.vector.tensor_add(t3[:], t1[:], t2[:])
                    nc.vector.tensor_mul(src[:], t3[:], invb[:])  # roped in place
                # v -> natural [t, d] tiles (into freed gates tile)
                for j in range(S // P):
                    tp = pcps.tile([P, P], BF, tag="vtp")
                    nc.tensor.transpose(tp[:], qkvT[6][:, j * P:(j + 1) * P], identb[:])
                    nc.vector.tensor_copy(v_nat[:, j * P:(j + 1) * P], tp[:])

            # ---------------- phase D: attention ----------------
            rk = qkvT[NHL]
            with tc.tile_pool(name="phD", bufs=3) as pd, \
                 tc.tile_pool(name="phD_g", bufs=2) as pdg, \
                 tc.tile_pool(name="phD_o", bufs=2, space="PSUM") as pdo, \
                 tc.tile_pool(name="phD_s", bufs=2, space="PSUM") as pds, \
                 tc.tile_pool(name="phD_den", bufs=2, space="PSUM") as pdd, \
                 tc.tile_pool(name="phD_dram", bufs=3, space="DRAM") as pddr:
                for hh in range(NHL):
                    grow = pdg.tile([1, S], BF, tag="grow")
                    nc.sync.dma_start(grow[:], gsd[hh:hh + 1, :])
                    for ic in range(S // 512):
                        isl = slice(ic * 512, (ic + 1) * 512)
                        po = pdo.tile([P, 512], F32, tag="po")
                        pden = pdd.tile([1, 512], F32, tag="pden")
                        for j in range(S // P):
                            psc = pds.tile([P, 512], F32, tag="psc")
                            nc.tensor.matmul(psc[:], rk[:, j * P:(j + 1) * P],
                                             qkvT[hh][:, isl], start=True, stop=True)
                            es = pd.tile([P, 512], BF, tag="es")
                            nc.scalar.activation(es[:], psc[:], AF.Exp)
                            nc.tensor.matmul(po[:], v_nat[:, j * P:(j + 1) * P], es[:],
                                             start=(j == 0), stop=(j == S // P - 1))
                            nc.tensor.matmul(pden[:], ones_b[:], es[:],
                                             start=(j == 0), stop=(j == S // P - 1))
                        rden = pd.tile([1, 512], F32, tag="rden")
                        nc.vector.reciprocal(rden[:], pden[:])
                        frow = pd.tile([1, 512], BF, tag="frow")
                        nc.vector.tensor_mul(frow[:], rden[:], grow[0:1, isl])
                        facd = pddr.tile([1, 512], BF, tag="facd")
                        nc.sync.dma_start(facd[:], frow[:])
                        facb = pd.tile([P, 512], BF, tag="facb")
                        nc.sync.dma_start(facb[:], facd[0:1, :].to_broadcast([P, 512]))
                        oev = pd.tile([P, 512], BF, tag="oev")
                        nc.vector.tensor_copy(oev[:], po[:])
                        # qkvT[hh][:, isl] (roped q) is dead after its j-loop
                        nc.vector.tensor_mul(ogt[hh][:, isl], oev[:], facb[:])

            # ---------------- phase E: routed proj + ReduceScatter ----------
            with tc.tile_pool(name="phE_w", bufs=2) as pew, \
                 tc.tile_pool(name="phE", bufs=4) as pe, \
                 tc.tile_pool(name="phE_h", bufs=2) as peh, \
                 tc.tile_pool(name="phE_ps", bufs=3, space="PSUM") as peps:
                holds = {}
                for oc in range(10):
                    ydst, ocol = (y0, oc * 512) if oc < 5 else (y1, (oc - 5) * 512)
                    for m in range(M):
                        lo, hi = off[m], off[m + 1]
                        pw = pew.tile([P, NHL * 512], BF, tag="pw")
                        nc.sync.dma_start(
                            pw[:].rearrange("p (hb f) -> p hb f", hb=NHL),
                            projw_ext[m, oc])
                        tc0, tc1 = lo // P, (hi - 1) // P
                        for tcx in range(tc0, tc1 + 1):
                            ps = peps.tile([P, 512], F32, tag="yps")
                            for hb in range(NHL):
                                nc.tensor.matmul(
                                    ps[:], ogt[hb][:, tcx * P:(tcx + 1) * P],
                                    pw[:, hb * 512:(hb + 1) * 512],
                                    start=(hb == 0), stop=(hb == NHL - 1))
                            if tcx in bnds:
                                bidx = bnds[tcx]
                                if m == bidx:  # lower mod: hold masked partial
                                    hv = peh.tile([P, 512], F32, tag=f"hold{bidx}")
                                    nc.vector.tensor_scalar_mul(
                                        hv[:], ps[:], bmask[:, 2 * bidx:2 * bidx + 1])
                                    holds[(oc, tcx)] = hv
                                else:  # upper mod: merge with inverse mask, write
                                    hv = holds.pop((oc, tcx))
                                    mg = pe.tile([P, 512], F32, tag="mg")
                                    nc.vector.tensor_scalar_mul(
                                        mg[:], ps[:],
                                        bmask[:, 2 * bidx + 1:2 * bidx + 2])
                                    yev = pe.tile([P, 512], BF, tag="yev")
                                    nc.vector.tensor_add(yev[:], hv[:], mg[:])
                                    nc.sync.dma_start(
                                        ydst[tcx * P:(tcx + 1) * P, ocol:ocol + 512],
                                        yev[:])
                            else:
                                yev = pe.tile([P, 512], BF, tag="yev")
                                nc.vector.tensor_copy(yev[:], ps[:])
                                nc.sync.dma_start(
                                    ydst[tcx * P:(tcx + 1) * P, ocol:ocol + 512],
                                    yev[:])
                nc.gpsimd.collective_compute(
                    "ReduceScatter", mybir.AluOpType.add, replica_groups=RG,
                    ins=[y0[:]], outs=[rs0[:]])
                nc.gpsimd.collective_compute(
                    "ReduceScatter", mybir.AluOpType.add, replica_groups=RG,
                    ins=[y1[:]], outs=[rs1[:]])
                nc.sync.dma_start(out_ext[0], rs0[:])
                nc.sync.dma_start(out_ext[1], rs1[:])

    _split_excess_waits(nc)
    return nc


# ---------------- host-side prep ----------------

def _prep_shards(x, rope_cos, rope_sin, modality_ids, pre_norm, qkv_w, q_norm,
                 k_norm, proj_w, perm, counts):
    """Build the 8 per-core in_maps (all host work is index/layout prep)."""
    bf16 = ml_dtypes.bfloat16
    mods = np.asarray(modality_ids).reshape(S)
    x2 = np.asarray(x).reshape(S, H)[perm]                      # sorted tokens
    cos = np.asarray(rope_cos).reshape(S, D)[perm]
    sin = np.asarray(rope_sin).reshape(S, D)[perm]
    mods_s = mods[perm]
    pn = np.asarray(pre_norm).reshape(M, H)
    qn = np.asarray(q_norm).reshape(M, D)
    kn = np.asarray(k_norm).reshape(M, D)
    qkv3 = np.asarray(qkv_w).reshape(M, QKV_PER_MOD, H)
    proj3 = np.asarray(proj_w).reshape(M, H, H)

    # rope tables [d, t]: roped = q*A + shift64(q)*B   (norm weights folded in)
    def rope_tables(nw_tok):
        # nw_tok: [S, D] per-token norm weight (sorted order)
        A = nw_tok * cos                                        # [S, D]
        Bm = np.empty_like(A)
        nw_sh = np.concatenate([nw_tok[:, 64:], nw_tok[:, :64]], axis=1)
        Bm[:, :64] = -nw_sh[:, :64] * sin[:, :64]
        Bm[:, 64:] = nw_sh[:, 64:] * sin[:, 64:]
        return A.T.astype(bf16), Bm.T.astype(bf16)              # [D, S]

    Aq, Bq = rope_tables(qn[mods_s])
    Ak, Bk = rope_tables(kn[mods_s])
    ropes = np.ascontiguousarray(np.stack([Aq, Bq, Ak, Bk]))    # [4, 128, S]

    # boundary masks for proj chunks
    off = [0, counts[0], counts[0] + counts[1], S]
    bmask = np.zeros((4, P, 1), np.float32)
    for b in (1, 2):
        if off[b] % P != 0:
            tcx = off[b] // P
            toks = tcx * P + np.arange(P)
            bmask[2 * (b - 1), :, 0] = (toks < off[b]).astype(np.float32)
            bmask[2 * (b - 1) + 1, :, 0] = (toks >= off[b]).astype(np.float32)

    in_maps = []
    for c in range(NCORE):
        # qkv rows for this core, pre_norm folded, transposed, tiled
        rq = qkv3[:, c * NHL * D:(c + 1) * NHL * D, :]          # [3, 640, H]
        rk = qkv3[:, Q_DIM + c * D:Q_DIM + (c + 1) * D, :]      # [3, 128, H]
        rv = qkv3[:, Q_DIM + K_DIM + c * D:Q_DIM + K_DIM + (c + 1) * D, :]
        rg = qkv3[:, Q_DIM + 2 * K_DIM + c * NHL:Q_DIM + 2 * K_DIM + (c + 1) * NHL, :]
        Wm = np.concatenate(
            [rq, rk, rv, rg, np.zeros((M, 1024 - 901, H), np.float32)], axis=1)
        Wm = Wm * pn[:, None, :]                                # fold pre_norm
        WT = Wm.transpose(0, 2, 1)                              # [3, H, 1024]
        qkvw = WT.reshape(M, HB, P, 8, P).transpose(0, 3, 2, 1, 4)  # [3,8,128,40,128]
        qkvw = np.ascontiguousarray(qkvw).astype(bf16)

        PT = proj3[:, :, c * NHL * D:(c + 1) * NHL * D].transpose(0, 2, 1)  # [3,640,H]
        projw = PT.reshape(M, NHL, P, 10, 512).transpose(0, 3, 2, 1, 4)
        projw = np.ascontiguousarray(projw).astype(bf16)        # [3,10,128,5,512]

        in_maps.append({
            "xs": np.ascontiguousarray(x2[c * SLOC:(c + 1) * SLOC]).astype(np.float32),
            "qkvw": qkvw,
            "projw": projw,
            "ropes": ropes,
            "bmask": bmask,
        })
    return in_maps


_CACHE = {}


def _get_module(counts, segw):
    key = (tuple(counts), segw)
    if key not in _CACHE:
        _CACHE[key] = build_module(counts, segw)
    return _CACHE[key]


def kernel(x, rope_cos, rope_sin, modality_ids, pre_norm, qkv_w, q_norm,
           k_norm, proj_w):
    mods = np.asarray(modality_ids).reshape(S)
    perm = np.argsort(mods, kind="stable")
    counts = [int((mods == m).sum()) for m in range(M)]
    assert min(counts) >= P, "modality segments must span at least one chunk"
    segw = min(((max(counts) + 63) // 64) * 64 + 64, S)
    nc = _get_module(counts, segw)
    in_maps = _prep_shards(x, rope_cos, rope_sin, modality_ids, pre_norm,
                           qkv_w, q_norm, k_norm, proj_w, perm, counts)
    res = run_bass_kernel_spmd(nc, in_maps, list(range(NCORE)), trace=False)
    y_sorted = np.empty((S, H), np.float32)
    for c in range(NCORE):
        o = res.results[c]["out"].astype(np.float32)            # [2, 256, 2560]
        y_sorted[c * SLOC:(c + 1) * SLOC, :2560] = o[0]
        y_sorted[c * SLOC:(c + 1) * SLOC, 2560:] = o[1]
    y = np.empty_like(y_sorted)
    y[perm] = y_sorted
    return y[None]


# lazy import so kernel.py can be imported without the runtime for inspection
from concourse.bass_utils import run_bass_kernel_spmd  # noqa: E402


# revision 12
# speedup vs baseline: 1.0597x; 1.0597x over previous
"""Trainium2 8-core kernel for modality-routed attention (nn_Attention_21715354648747).

Strategy (per sharding hint + modality-sorted routing):
- Host: sort tokens by modality; fold pre_norm into qkv_w, fold q/k_norm+rope
  into per-token tables; pre-tile weights per core (column-split QKV: 5 Q heads
  + 1 KV group + gates per core; row-split proj over the core's 640 head dims).
- Device: seq-parallel RMSNorm + transpose -> AllGather(bf16) -> routed QKV
  (weights stationary, per-modality contiguous token segments) -> QK norm +
  rope in [d,t] layout -> attention per head (scores^T, exp, PV + denominator
  matmuls) -> gated, 1/denom scaled -> routed proj in column halves with
  early ReduceScatter(bf16) overlap.
- Host: concat shards, invert permutation, cast f32.
"""
import sys

for _p in ("/opt/trn_rl_repo",):
    if _p not in sys.path:
        sys.path.append(_p)

import numpy as np
import ml_dtypes

import concourse.bass as bass
import concourse.tile as tile
from concourse import mybir
from concourse.masks import make_identity

# ---------------- problem constants (hardcoded) ----------------
S = 2048
H = 5120
HB = H // 128          # 40 h-blocks
D = 128
NCORE = 8
SLOC = S // NCORE      # 256 tokens per core
NQH = 40
NHL = NQH // NCORE     # 5 q heads per core
NKV = 8
M = 3
QKV_PER_MOD = NQH * D + 2 * NKV * D + NQH  # 7208
Q_DIM = NQH * D        # 5120
K_DIM = NKV * D        # 1024
EPS = 1e-6

DEBUG = False

BF = mybir.dt.bfloat16
F32 = mybir.dt.float32
AF = mybir.ActivationFunctionType
P = 128

_MAX_WAITS = 1
_wsplit_counter = [0]


def _split_excess_waits(nc, max_waits=_MAX_WAITS):
    """This walrus build encodes at most one sync wait per instruction; Tile's
    wait coalescing (notably the kernel-tail Drain) can exceed that. Move the
    excess waits onto NOPs inserted immediately before, on the same engine."""
    for fn in nc.m.functions:
        for bb in fn.blocks:
            il = bb.instructions
            snapshot = list(il)
            new_list = []
            changed = False
            for ins in snapshot:
                si = ins.sync_info
                waits = list(si.on_wait) if si is not None else []
                if len(waits) > max_waits:
                    extra = waits[: len(waits) - max_waits]
                    keep = waits[len(waits) - max_waits:]
                    for c in range(0, len(extra), max_waits):
                        chunk = extra[c:c + max_waits]
                        _wsplit_counter[0] += 1
                        nop = mybir.InstNoOp(
                            name=f"Wsplit-{_wsplit_counter[0]}", ins=[], outs=[]
                        )
                        nop.engine = ins.engine
                        nop.sync_info = mybir.SyncInfo(on_wait=chunk, on_update=[])
                        new_list.append(nop)
                        changed = True
                    si.on_wait[:] = keep
                new_list.append(ins)
            if changed:
                il[:] = new_list


def build_module(counts):
    """Build the SPMD Bass module for given modality counts (token-sorted)."""
    c0, c1, c2 = counts
    off = [0, c0, c0 + c1, S]
    nc = bass.Bass()

    # ---- DRAM parameters (per-core shards via in_maps) ----
    xs_ext = nc.declare_dram_parameter("xs", [SLOC, H], F32, isOutput=False)
    qkvw_ext = nc.declare_dram_parameter("qkvw", [M, 8, P, HB, P], BF, isOutput=False)
    projw_ext = nc.declare_dram_parameter("projw", [M, 10, P, NHL, 512], BF, isOutput=False)
    ropes_ext = nc.declare_dram_parameter("ropes", [4, P, S], BF, isOutput=False)
    bmask_ext = nc.declare_dram_parameter("bmask", [4, P, 1], F32, isOutput=False)
    out_ext = nc.declare_dram_parameter("out", [2, SLOC, 2560], BF, isOutput=True)
    if DEBUG:
        dbg_ag = nc.declare_dram_parameter("dbg_ag", [NCORE, P, HB * SLOC], BF, isOutput=True)
        dbg_xb = nc.declare_dram_parameter("dbg_xb", [M, 4, P, HB * SLOC], BF, isOutput=True)
        dbg_qkv = nc.declare_dram_parameter("dbg_qkv", [8, P, S], BF, isOutput=True)
        dbg_rope = nc.declare_dram_parameter("dbg_rope", [6, P, S], BF, isOutput=True)
        dbg_vnat = nc.declare_dram_parameter("dbg_vnat", [P, S], BF, isOutput=True)
        dbg_ogt = nc.declare_dram_parameter("dbg_ogt", [NHL, P, S], BF, isOutput=True)
        dbg_gs = nc.declare_dram_parameter("dbg_gs", [NHL, S], BF, isOutput=True)

    # ---- internal DRAM (partition-major gather layout) ----
    agin = nc.dram_tensor("agin", [P, HB, SLOC], BF)
    agout = nc.dram_tensor("agout", [NCORE, P, HB, SLOC], BF, addr_space="Shared")
    gsd = nc.dram_tensor("gsd", [NHL, S], BF)
    yh = [nc.dram_tensor(f"y{i}", [S, 2560], BF) for i in range(2)]
    rsh = [nc.dram_tensor(f"rs{i}", [SLOC, 2560], BF) for i in range(2)]

    RG = [list(range(NCORE))]

    # boundary chunks for proj (tokens on partitions, chunks of 128)
    bnds = {}  # tc -> boundary idx (0: between mod0/1, 1: between mod1/2)
    for b in (1, 2):
        if off[b] % P != 0:
            bnds[off[b] // P] = b - 1

    # per-mod QKV matmul sub-chunks at rank-block boundaries
    def qkv_chunks(m):
        lo, hi = off[m], off[m + 1]
        out = []  # (rank block, col lo, col hi)  [block-local]
        t = lo
        while t < hi:
            r = t // SLOC
            nxt = min(hi, (r + 1) * SLOC)
            out.append((r, t - r * SLOC, nxt - r * SLOC))
            t = nxt
        return out

    with tile.TileContext(nc) as tc:
        with tc.tile_pool(name="const", bufs=1) as constp, \
             tc.tile_pool(name="resident", bufs=1) as resp:
            identb = constp.tile([P, P], BF)
            make_identity(nc, identb[:])
            ones_b = constp.tile([P, 1], BF)
            nc.vector.memset(ones_b[:], 1.0)
            ropes = constp.tile([P, 4 * S], BF)
            nc.sync.dma_start(
                ropes[:].rearrange("p (a f) -> p a f", a=4),
                ropes_ext.rearrange("a p f -> p a f"))
            bmask = constp.tile([P, 4], F32)
            nc.sync.dma_start(
                bmask[:].rearrange("p (a f) -> p a f", a=4),
                bmask_ext.rearrange("a p f -> p a f"))

            # qkvT resident tiles: 0-4 q heads, 5 k, 6 v, 7 gates (then v_nat)
            qkvT = [resp.tile([P, S], BF, tag=f"qkvT{i}", name=f"qkvT{i}")
                    for i in range(8)]
            v_nat = qkvT[7]   # reused after gates move to DRAM
            ogt = qkvT[:NHL]  # roped q overwritten by gated attn out slices

            # ---------------- phase A: pre-norm + transpose + AllGather ----
            with tc.tile_pool(name="phA", bufs=2) as pa, \
                 tc.tile_pool(name="phA_ps", bufs=3, space="PSUM") as paps:
                staged = pa.tile([P, HB * SLOC], BF, tag="staged")
                for tt in range(SLOC // P):
                    xt = pa.tile([P, H], F32, tag="xt")
                    nc.sync.dma_start(xt[:], xs_ext[tt * P:(tt + 1) * P, :])
                    sq = pa.tile([P, H], BF, tag="sq")
                    ssq = pa.tile([P, 1], F32, tag="ssq")
                    nc.scalar.activation(sq[:], xt[:], AF.Square, accum_out=ssq[:])
                    z = pa.tile([P, 1], F32, tag="z")
                    nc.vector.tensor_scalar(z[:], ssq[:], 1.0 / H, EPS,
                                            mybir.AluOpType.mult, mybir.AluOpType.add)
                    zr = pa.tile([P, 1], F32, tag="zr")
                    nc.vector.reciprocal(zr[:], z[:])
                    inv = pa.tile([P, 1], F32, tag="inv")
                    nc.scalar.activation(inv[:], zr[:], AF.Sqrt)
                    xh = pa.tile([P, H], BF, tag="xh")
                    nc.vector.tensor_scalar_mul(xh[:], xt[:], inv[:])
                    for hb in range(HB):
                        tp = paps.tile([P, P], BF, tag="tp")
                        nc.tensor.transpose(tp[:], xh[:, hb * P:(hb + 1) * P], identb[:])
                        nc.vector.tensor_copy(
                            staged[:, hb * SLOC + tt * P:hb * SLOC + (tt + 1) * P],
                            tp[:])
                nc.sync.dma_start(agin[:], staged[:])
            nc.gpsimd.collective_compute(
                "AllGather", mybir.AluOpType.bypass, replica_groups=RG,
                ins=[agin[:]], outs=[agout[:]])
            if DEBUG:
                nc.sync.dma_start(
                    dbg_ag[:].rearrange("r p f -> r p f"),
                    agout.rearrange("r p hb f -> r p (hb f)"))

            # ------- phases B/C/D interleaved: QKV, norms+rope, attention ----
            OT_ORDER = [7, 6, 5, 0, 1, 2, 3, 4]
            with tc.tile_pool(name="phB_x", bufs=1) as pbx, \
                 tc.tile_pool(name="phB_w", bufs=2) as pbw, \
                 tc.tile_pool(name="phC", bufs=1) as pc, \
                 tc.tile_pool(name="phCg", bufs=2) as pcg, \
                 tc.tile_pool(name="phD", bufs=2) as pd, \
                 tc.tile_pool(name="phB_ps", bufs=2, space="PSUM") as pbps, \
                 tc.tile_pool(name="phC_ps", bufs=1, space="PSUM") as pcps, \
                 tc.tile_pool(name="phD_s", bufs=2, space="PSUM") as pds, \
                 tc.tile_pool(name="phD_o", bufs=1, space="PSUM") as pdo, \
                 tc.tile_pool(name="phD_den", bufs=1, space="PSUM") as pdd, \
                 tc.tile_pool(name="dramp", bufs=3, space="DRAM") as drp:

                def phase_b(m, ot, xblks):
                    chunks = qkv_chunks(m)
                    wbuf = pbw.tile([P, HB * P], BF, tag="wbuf", name="wbuf")
                    nc.sync.dma_start(
                        wbuf[:].rearrange("p (hb f) -> p hb f", hb=HB),
                        qkvw_ext[m, ot])
                    # one PSUM tile per chunk: start=True clears has_written at
                    # bank granularity, so accumulation chunks must not share a
                    # live bank
                    for (r, cl, ch) in chunks:
                        w = ch - cl
                        ps = pbps.tile([P, SLOC], F32, tag="qkvps", name="qkvps")
                        for hb in range(HB):
                            nc.tensor.matmul(
                                ps[:, :w],
                                wbuf[:, hb * P:(hb + 1) * P],
                                xblks[r][:, hb * SLOC + cl:hb * SLOC + ch],
                                start=(hb == 0), stop=(hb == HB - 1))
                        nc.vector.tensor_copy(
                            qkvT[ot][:, r * SLOC + cl:r * SLOC + ch], ps[:, :w])

                def phase_c(kk):
                    src = qkvT[kk]
                    is_q = kk < NHL
                    sq = pc.tile([P, S], BF, tag="csq", name="csq")
                    nc.scalar.activation(sq[:], src[:], AF.Square)
                    invrow = pc.tile([1, S], BF, tag="invrow", name="invrow")
                    for ic in range(S // 512):
                        ssp = pcps.tile([1, 512], F32, tag="ssp", name="ssp")
                        nc.tensor.matmul(ssp[:], ones_b[:],
                                         sq[:, ic * 512:(ic + 1) * 512],
                                         start=True, stop=True)
                        z = pc.tile([1, 512], F32, tag="cz", name="cz")
                        if is_q:
                            # fold 1/sqrt(D): rsqrt(ssq + D*eps)
                            nc.vector.tensor_scalar_add(z[:], ssp[:], D * EPS)
                        else:
                            nc.vector.tensor_scalar(z[:], ssp[:], 1.0 / D, EPS,
                                                    mybir.AluOpType.mult,
                                                    mybir.AluOpType.add)
                        zr = pc.tile([1, 512], F32, tag="czr", name="czr")
                        nc.vector.reciprocal(zr[:], z[:])
                        nc.scalar.activation(invrow[:, ic * 512:(ic + 1) * 512],
                                             zr[:], AF.Sqrt)
                    invd = drp.tile([1, S], BF, tag="invd", name="invd")
                    nc.sync.dma_start(invd[:], invrow[:])
                    invb = pc.tile([P, S], BF, tag="invb", name="invb")
                    nc.sync.dma_start(invb[:], invd[0:1, :].to_broadcast([P, S]))
                    sh = pc.tile([P, S], BF, tag="csh", name="csh")
                    nc.sync.dma_start(sh[0:64, :], src[64:128, :])
                    nc.sync.dma_start(sh[64:128, :], src[0:64, :])
                    A = ropes[:, (0 if is_q else 2) * S:(1 if is_q else 3) * S]
                    B = ropes[:, (1 if is_q else 3) * S:(2 if is_q else 4) * S]
                    t1 = pc.tile([P, S], BF, tag="ct1", name="ct1")
                    nc.vector.tensor_mul(t1[:], src[:], A)
                    t2 = pc.tile([P, S], BF, tag="ct2", name="ct2")
                    nc.vector.tensor_mul(t2[:], sh[:], B)
                    nc.vector.tensor_add(t1[:], t1[:], t2[:])
                    nc.vector.tensor_mul(src[:], t1[:], invb[:])  # roped in place

                def phase_d(hh):
                    rk = qkvT[NHL]
                    grow = pcg.tile([1, S], BF, tag="grow", name="grow")
                    nc.sync.dma_start(grow[:], gsd[hh:hh + 1, :])
                    for ic in range(S // 512):
                        isl = slice(ic * 512, (ic + 1) * 512)
                        po = pdo.tile([P, 512], F32, tag="po", name="po")
                        pden = pdd.tile([1, 512], F32, tag="pden", name="pden")
                        for j in range(S // P):
                            psc = pds.tile([P, 512], F32, tag="psc", name="psc")
                            nc.tensor.matmul(psc[:], rk[:, j * P:(j + 1) * P],
                                             qkvT[hh][:, isl], start=True, stop=True)
                            es = pd.tile([P, 512], BF, tag="es", name="es")
                            nc.scalar.activation(es[:], psc[:], AF.Exp)
                            nc.tensor.matmul(po[:], v_nat[:, j * P:(j + 1) * P], es[:],
                                             start=(j == 0), stop=(j == S // P - 1))
                            nc.tensor.matmul(pden[:], ones_b[:], es[:],
                                             start=(j == 0), stop=(j == S // P - 1))
                        rden = pd.tile([1, 512], F32, tag="rden", name="rden")
                        nc.vector.reciprocal(rden[:], pden[:])
                        frow = pd.tile([1, 512], BF, tag="frow", name="frow")
                        nc.vector.tensor_mul(frow[:], rden[:], grow[0:1, isl])
                        facd = drp.tile([1, 512], BF, tag="facd", name="facd")
                        nc.sync.dma_start(facd[:], frow[:])
                        facb = pd.tile([P, 512], BF, tag="facb", name="facb")
                        nc.sync.dma_start(facb[:], facd[0:1, :].to_broadcast([P, 512]))
                        oev = pd.tile([P, 512], BF, tag="oev", name="oev")
                        nc.vector.tensor_copy(oev[:], po[:])
                        # qkvT[hh][:, isl] (roped q) is dead after its j-loop
                        nc.vector.tensor_mul(ogt[hh][:, isl], oev[:], facb[:])

                def load_xblks(m):
                    lo, hi = off[m], off[m + 1]
                    r0, r1 = lo // SLOC, (hi - 1) // SLOC
                    xblks = {}
                    for i, r in enumerate(range(r0, r1 + 1)):
                        xb = pbx.tile([P, HB * SLOC], BF, tag=f"xblk{i}",
                                      name=f"xblk{i}")
                        nc.sync.dma_start(
                            xb[:].rearrange("p (hb f) -> p hb f", hb=HB),
                            agout[r])
                        if DEBUG:
                            nc.sync.dma_start(dbg_xb[m, i], xb[:])
                        xblks[r] = xb
                    return xblks

                for m in range(2):
                    xblks = load_xblks(m)
                    for ot in OT_ORDER:
                        phase_b(m, ot, xblks)
                xblks = load_xblks(2)
                for ot in OT_ORDER:
                    phase_b(2, ot, xblks)
                    if DEBUG:
                        nc.sync.dma_start(dbg_qkv[ot], qkvT[ot][:])
                    if ot == 7:
                        grows = pc.tile([NHL, S], BF, tag="grows", name="grows")
                        nc.scalar.activation(grows[:], qkvT[7][0:NHL, :], AF.Sigmoid)
                        nc.sync.dma_start(gsd[:], grows[:])
                    elif ot == 6:
                        pass  # v ready; transposed after gates leave qkvT[7]
                    elif ot == 5:
                        for j in range(S // P):
                            tp = pcps.tile([P, P], BF, tag="vtp", name="vtp")
                            nc.tensor.transpose(tp[:], qkvT[6][:, j * P:(j + 1) * P],
                                                identb[:])
                            nc.vector.tensor_copy(v_nat[:, j * P:(j + 1) * P], tp[:])
                        if DEBUG:
                            nc.sync.dma_start(dbg_vnat[:], v_nat[:])
                        phase_c(NHL)  # k
                        if DEBUG:
                            nc.sync.dma_start(dbg_rope[NHL], qkvT[NHL][:])
                            nc.sync.dma_start(dbg_gs[:], gsd[:])
                    else:
                        phase_c(ot)
                        if DEBUG:
                            nc.sync.dma_start(dbg_rope[ot], qkvT[ot][:])
                        phase_d(ot)
                        if DEBUG:
                            nc.sync.dma_start(dbg_ogt[ot], ogt[ot][:])

            # ---------------- phase E: routed proj + ReduceScatter ----------
            with tc.tile_pool(name="phE_w", bufs=2) as pew, \
                 tc.tile_pool(name="phE", bufs=3) as pe, \
                 tc.tile_pool(name="phE_h", bufs=1) as peh, \
                 tc.tile_pool(name="phE_ps", bufs=3, space="PSUM") as peps:
                for half in range(2):
                    holds = {}
                    for m in range(M):
                        lo, hi = off[m], off[m + 1]
                        pw = pew.tile([P, 5 * NHL * 512], BF, tag="pw", name="pw")
                        nc.sync.dma_start(
                            pw[:].rearrange("p (oc hb f) -> p oc hb f", oc=5, hb=NHL),
                            projw_ext[m, half * 5:(half + 1) * 5].rearrange(
                                "oc p hb f -> p oc hb f"))
                        tc0, tc1 = lo // P, (hi - 1) // P
                        for tcx in range(tc0, tc1 + 1):
                            ystage = pe.tile([P, 2560], BF, tag="ystage",
                                             name="ystage")
                            for ol in range(5):
                                ps = peps.tile([P, 512], F32, tag="yps", name="yps")
                                for hb in range(NHL):
                                    nc.tensor.matmul(
                                        ps[:], ogt[hb][:, tcx * P:(tcx + 1) * P],
                                        pw[:, (ol * NHL + hb) * 512:
                                           (ol * NHL + hb + 1) * 512],
                                        start=(hb == 0), stop=(hb == NHL - 1))
                                nc.vector.tensor_copy(
                                    ystage[:, ol * 512:(ol + 1) * 512], ps[:])
                            if tcx in bnds:
                                bidx = bnds[tcx]
                                if m == bidx:  # lower mod: hold masked partial
                                    hv = peh.tile([P, 2560], BF, tag=f"hold{bidx}",
                                                  name=f"hold{bidx}")
                                    nc.vector.tensor_scalar_mul(
                                        hv[:], ystage[:],
                                        bmask[:, 2 * bidx:2 * bidx + 1])
                                    holds[tcx] = hv
                                else:  # upper mod: merge with inverse mask
                                    hv = holds.pop(tcx)
                                    mg = pe.tile([P, 2560], BF, tag="mg", name="mg")
                                    nc.vector.tensor_scalar_mul(
                                        mg[:], ystage[:],
                                        bmask[:, 2 * bidx + 1:2 * bidx + 2])
                                    yo = pe.tile([P, 2560], BF, tag="yo", name="yo")
                                    nc.vector.tensor_add(yo[:], hv[:], mg[:])
                                    nc.sync.dma_start(
                                        yh[half][tcx * P:(tcx + 1) * P, :], yo[:])
                            else:
                                nc.sync.dma_start(
                                    yh[half][tcx * P:(tcx + 1) * P, :], ystage[:])
                    nc.gpsimd.collective_compute(
                        "ReduceScatter", mybir.AluOpType.add, replica_groups=RG,
                        ins=[yh[half][:]], outs=[rsh[half][:]])
                    nc.sync.dma_start(out_ext[half], rsh[half][:])

    _split_excess_waits(nc)
    return nc


# ---------------- host-side prep ----------------

def _prep_shards(x, rope_cos, rope_sin, modality_ids, pre_norm, qkv_w, q_norm,
                 k_norm, proj_w, perm, counts):
    """Build the 8 per-core in_maps (host work is index/layout prep only)."""
    bf16 = ml_dtypes.bfloat16
    mods = np.asarray(modality_ids).reshape(S)
    x2 = np.asarray(x).reshape(S, H)[perm]                      # sorted tokens
    cos = np.asarray(rope_cos).reshape(S, D)[perm]
    sin = np.asarray(rope_sin).reshape(S, D)[perm]
    mods_s = mods[perm]
    pn = np.asarray(pre_norm).reshape(M, H)
    qn = np.asarray(q_norm).reshape(M, D)
    kn = np.asarray(k_norm).reshape(M, D)
    qkv3 = np.asarray(qkv_w).reshape(M, QKV_PER_MOD, H)
    proj3 = np.asarray(proj_w).reshape(M, H, H)

    # rope tables [d, t]: roped = q*A + shift64(q)*B   (norm weights folded in)
    def rope_tables(nw_tok):
        A = nw_tok * cos                                        # [S, D]
        Bm = np.empty_like(A)
        nw_sh = np.concatenate([nw_tok[:, 64:], nw_tok[:, :64]], axis=1)
        Bm[:, :64] = -nw_sh[:, :64] * sin[:, :64]
        Bm[:, 64:] = nw_sh[:, 64:] * sin[:, 64:]
        return A.T.astype(bf16), Bm.T.astype(bf16)              # [D, S]

    Aq, Bq = rope_tables(qn[mods_s])
    Ak, Bk = rope_tables(kn[mods_s])
    ropes = np.ascontiguousarray(np.stack([Aq, Bq, Ak, Bk]))    # [4, 128, S]

    # boundary masks for proj chunks
    off = [0, counts[0], counts[0] + counts[1], S]
    bmask = np.zeros((4, P, 1), np.float32)
    for b in (1, 2):
        if off[b] % P != 0:
            tcx = off[b] // P
            toks = tcx * P + np.arange(P)
            bmask[2 * (b - 1), :, 0] = (toks < off[b]).astype(np.float32)
            bmask[2 * (b - 1) + 1, :, 0] = (toks >= off[b]).astype(np.float32)

    in_maps = []
    for c in range(NCORE):
        rq = qkv3[:, c * NHL * D:(c + 1) * NHL * D, :]          # [3, 640, H]
        rk = qkv3[:, Q_DIM + c * D:Q_DIM + (c + 1) * D, :]      # [3, 128, H]
        rv = qkv3[:, Q_DIM + K_DIM + c * D:Q_DIM + K_DIM + (c + 1) * D, :]
        rg = qkv3[:, Q_DIM + 2 * K_DIM + c * NHL:Q_DIM + 2 * K_DIM + (c + 1) * NHL, :]
        Wm = np.concatenate(
            [rq, rk, rv, rg, np.zeros((M, 1024 - 901, H), np.float32)], axis=1)
        Wm = Wm * pn[:, None, :]                                # fold pre_norm
        WT = Wm.transpose(0, 2, 1)                              # [3, H, 1024]
        qkvw = WT.reshape(M, HB, P, 8, P).transpose(0, 3, 2, 1, 4)
        qkvw = np.ascontiguousarray(qkvw).astype(bf16)          # [3,8,128,40,128]

        PT = proj3[:, :, c * NHL * D:(c + 1) * NHL * D].transpose(0, 2, 1)
        projw = PT.reshape(M, NHL, P, 10, 512).transpose(0, 3, 2, 1, 4)
        projw = np.ascontiguousarray(projw).astype(bf16)        # [3,10,128,5,512]

        in_maps.append({
            "xs": np.ascontiguousarray(x2[c * SLOC:(c + 1) * SLOC]).astype(np.float32),
            "qkvw": qkvw,
            "projw": projw,
            "ropes": ropes,
            "bmask": bmask,
        })
    return in_maps


_CACHE = {}


def _get_module(counts):
    key = tuple(counts)
    if key not in _CACHE:
        _CACHE[key] = build_module(counts)
    return _CACHE[key]


def kernel(x, rope_cos, rope_sin, modality_ids, pre_norm, qkv_w, q_norm,
           k_norm, proj_w):
    mods = np.asarray(modality_ids).reshape(S)
    perm = np.argsort(mods, kind="stable")
    counts = [int((mods == m).sum()) for m in range(M)]
    assert min(counts) >= P, "modality segments must span at least one chunk"
    nc = _get_module(counts)
    in_maps = _prep_shards(x, rope_cos, rope_sin, modality_ids, pre_norm,
                           qkv_w, q_norm, k_norm, proj_w, perm, counts)
    res = run_bass_kernel_spmd(nc, in_maps, list(range(NCORE)), trace=False)
    y_sorted = np.empty((S, H), np.float32)
    for c in range(NCORE):
        o = res.results[c]["out"].astype(np.float32)            # [2, 256, 2560]
        y_sorted[c * SLOC:(c + 1) * SLOC, :2560] = o[0]
        y_sorted[c * SLOC:(c + 1) * SLOC, 2560:] = o[1]
    y = np.empty_like(y_sorted)
    y[perm] = y_sorted
    return y[None]


from concourse.bass_utils import run_bass_kernel_spmd  # noqa: E402
